# revision 9
# baseline (speedup 1.0000x reference)
"""Two-layer GAT on 8 Trainium2 NeuronCores.

Strategy (graph/data parallel, dst-ownership):
- Host: add self-loops, sort nodes by in-degree (desc), pad to 50176 nodes =
  392 blocks of 128; block b -> core b%8 so per-core degree distribution is
  balanced; new node ids are core-major so each core owns a contiguous range.
- Edges routed to the dst-owner core, stored as a padded ELL structure per
  128-dst block (degree sorting keeps padding ~10%).  Edge slots are split
  into "low"/"high" halves by src table id (dma_gather indices are int16).
- Device: per layer, a dense phase computes table rows [h | att_src] for all
  nodes (redundantly on every core) with one augmented matmul; the gather
  phase dma_gathers the per-edge src rows, computes the segment softmax
  (no max-subtraction needed: logits are O(10)) and aggregates with
  identity-weight matmuls accumulating in PSUM; normalization by the segment
  denominator happens once per dst block after aggregation.
- Between layers the per-core x2 shards (already transposed on device) are
  AllGathered.  Final log_softmax on device; host inverse-permutes rows.

Self-contained: only needs numpy + the concourse (bass) runtime.
"""

import numpy as np

import concourse.bass as bass
import concourse.mybir as mybir
import concourse.tile as tile
from concourse import bacc
from concourse.bass_utils import run_bass_kernel_spmd

# problem constants (hardcoded per spec nn_GAT_19318762897898)
N = 50000
IN_F = 32
HF = 32
OUTF = 16
HEADS = 8
NEG = 0.2
EPS = 1e-16

NCORE = 8
P = 128
BPC = 49                  # blocks per core
NPC = BPC * P             # 6272 nodes per core
NPAD = NCORE * NPC        # 50176
NBLK = NPAD // P          # 392
D1 = HEADS * HF           # 256
D2 = HEADS * OUTF         # 128
T1C = 320                 # table1 row, f32 (1280B: [h 256 | as 8 | pad])
T2C = 192                 # table2 row, f32 (768B:  [h2 128 | as2 8 | pad])
HIBASE = 32768
NROWS = NPAD + 2          # row 0: low dummy; row NPAD+1: high dummy
DUMHI = NPAD + 1 - HIBASE
BIGNEG = -1.0e9
CH = 8                    # gather chunk size in 128-slot tiles

F32 = mybir.dt.float32
I16 = mybir.dt.int16


def _preprocess(x, edge_index):
    """Permute + pack nodes, build per-core ELL gather indices.

    Two independent node labelings:
    - table position `pos` (0..NPAD-1): table row id = 1+pos; the int16
      low/high gather split is pos <= 32766.  Low/high membership is fixed
      first (by degree rank), then nodes are sorted within each half by
      their (low,high) in-edge counts so each 128-dst block has homogeneous
      counts (tight ELL padding).
    - ownership id `new_id` (core-major): blocks of 128 positions are dealt
      to (core, local-block) slots sorted by their (K_lo, K_hi) profile so
      the SPMD-uniform per-block tile counts stay tight across cores.
    """
    src0 = np.asarray(edge_index[0], dtype=np.int64)
    dst0 = np.asarray(edge_index[1], dtype=np.int64)
    loops = np.arange(N, dtype=np.int64)
    src = np.concatenate([src0, loops])
    dst = np.concatenate([dst0, loops])

    # stage 1: low/high membership by degree rank
    deg = np.bincount(dst, minlength=N)
    rank_of_old = np.empty(N, dtype=np.int64)
    rank_of_old[np.argsort(-deg, kind="stable")] = np.arange(N)
    is_lo = rank_of_old <= (HIBASE - 2)               # table id 1+pos <= 32767

    cl = np.bincount(dst[is_lo[src]], minlength=N)    # per-dst low in-edges
    chh = np.bincount(dst[~is_lo[src]], minlength=N)

    # stage 2: sort within each half by (cl, ch) desc -> table positions
    lo_nodes = np.flatnonzero(is_lo)
    hi_nodes = np.flatnonzero(~is_lo)
    lo_sorted = lo_nodes[np.lexsort((-chh[lo_nodes], -cl[lo_nodes]))]
    hi_sorted = hi_nodes[np.lexsort((-chh[hi_nodes], -cl[hi_nodes]))]
    pos_of_old = np.empty(N, dtype=np.int64)
    pos_of_old[lo_sorted] = np.arange(len(lo_sorted))
    pos_of_old[hi_sorted] = (HIBASE - 1) + np.arange(len(hi_sorted))

    # per-position counts; per-block maxima
    clp = np.zeros(NPAD, np.int64)
    chp = np.zeros(NPAD, np.int64)
    clp[pos_of_old] = cl
    chp[pos_of_old] = chh
    maxlo_g = clp.reshape(NBLK, P).max(1)             # [392]
    maxhi_g = chp.reshape(NBLK, P).max(1)

    # deal blocks (sorted by profile) to (core, lb) slots
    gsort = np.lexsort((-maxhi_g, -maxlo_g))          # dealt rank s -> block g
    core_of_g = np.empty(NBLK, np.int64)
    lb_of_g = np.empty(NBLK, np.int64)
    core_of_g[gsort] = np.arange(NBLK) % NCORE
    lb_of_g[gsort] = np.arange(NBLK) // NCORE
    # gpos[r][m] = table block of core r's local block m
    gpos = np.empty((NCORE, BPC), np.int64)
    gpos[core_of_g, lb_of_g] = np.arange(NBLK)

    # ownership ids
    g_of_old = pos_of_old // P
    j_of_old = pos_of_old % P
    new_of_old = core_of_g[g_of_old] * NPC + lb_of_g[g_of_old] * P + j_of_old

    x = np.asarray(x, dtype=np.float32)
    x_pad = np.zeros((NPAD, IN_F), np.float32)
    x_pad[new_of_old] = x

    tid = 1 + pos_of_old[src]                         # table row ids, 1..NPAD
    hi = tid > (HIBASE - 1)
    dst_n = new_of_old[dst]                           # ownership id of dst

    # rank of each edge within its (dst, half) group
    key = dst_n * 2 + hi
    o = np.argsort(key, kind="stable")
    ks = key[o]
    newrun = np.r_[True, ks[1:] != ks[:-1]]
    run_start = np.flatnonzero(newrun)
    run_len = np.diff(np.r_[run_start, len(ks)])
    rank_sorted = np.arange(len(ks), dtype=np.int64) - np.repeat(run_start, run_len)
    rank = np.empty(len(ks), np.int64)
    rank[o] = rank_sorted

    # uniform per-local-block tile counts (max over the 8 dealt blocks)
    K_lo = np.zeros(BPC, np.int64)
    K_hi = np.zeros(BPC, np.int64)
    for s in range(NBLK):
        g = gsort[s]
        lb = s // NCORE
        K_lo[lb] = max(K_lo[lb], maxlo_g[g])
        K_hi[lb] = max(K_hi[lb], maxhi_g[g])

    blk_tiles = K_lo + K_hi
    blk_off = np.r_[0, np.cumsum(blk_tiles)[:-1]]
    s_tiles = int(blk_tiles.sum())
    slots = s_tiles * P

    # defaults: low tiles gather row 0, high tiles gather row NPAD+1
    tile_is_hi = np.zeros(s_tiles, dtype=bool)
    for lb in range(BPC):
        t0 = blk_off[lb]
        tile_is_hi[t0 + K_lo[lb]: t0 + K_lo[lb] + K_hi[lb]] = True
    default = np.where(tile_is_hi, DUMHI, 0).astype(np.int16)
    base = np.repeat(default, P)                      # [slots]

    core_e = dst_n // NPC
    lb_e = (dst_n % NPC) // P
    j_e = dst_n % P
    t_in_blk = np.where(hi, K_lo[lb_e] + rank, rank)
    slot_e = (blk_off[lb_e] + t_in_blk) * P + j_e
    val_e = np.where(hi, tid - HIBASE, tid).astype(np.int16)

    idx_maps = []
    for c in range(NCORE):
        arr = base.copy()
        m = core_e == c
        arr[slot_e[m]] = val_e[m]
        wrapped = arr.reshape(-1, 16).T.copy()        # [16, slots/16]
        idx_maps.append(np.tile(wrapped, (NCORE, 1)))  # [128, slots/16]

    # xT stacked rank-major: [NCORE*IN_F, NPC]
    xT = np.ascontiguousarray(x_pad.T)                # [32, NPAD]
    xT_stack = np.concatenate(
        [xT[:, c * NPC:(c + 1) * NPC] for c in range(NCORE)], axis=0
    )                                                 # [256, NPC]

    sched = dict(K_lo=[int(v) for v in K_lo], K_hi=[int(v) for v in K_hi],
                 blk_off=[int(v) for v in blk_off], s_tiles=s_tiles, slots=slots,
                 gpos=[[int(v) for v in row] for row in gpos])
    return x_pad, xT_stack, idx_maps, new_of_old, sched


def _augment(W, att_src, att_dst, heads, out_ch):
    W = np.asarray(W, dtype=np.float32)
    a_s = np.asarray(att_src, dtype=np.float32)
    a_d = np.asarray(att_dst, dtype=np.float32)
    Wr = W.reshape(IN_F if W.shape[0] == IN_F else HF, heads, out_ch)
    was = np.einsum("khc,hc->kh", Wr, a_s)
    wad = np.einsum("khc,hc->kh", Wr, a_d)
    return np.concatenate([W, was, wad], axis=1).astype(np.float32)


def _build_nc(sched):
    K_lo = sched["K_lo"]
    K_hi = sched["K_hi"]
    blk_off = sched["blk_off"]
    s_tiles = sched["s_tiles"]
    gpos = sched["gpos"]

    nc = bacc.Bacc(None, target_bir_lowering=False, debug=False)

    xt_in = nc.declare_dram_parameter("xt", [NCORE * IN_F, NPC], F32, isOutput=False)
    xtown_in = nc.declare_dram_parameter("xtown", [IN_F, NPC], F32, isOutput=False)
    waug1_in = nc.declare_dram_parameter("waug1", [IN_F, D1 + 16], F32, isOutput=False)
    w2aug_in = nc.declare_dram_parameter("w2aug", [HF, D2 + 16], F32, isOutput=False)
    ident_in = nc.declare_dram_parameter("ident", [P, P], F32, isOutput=False)
    b1rep_in = nc.declare_dram_parameter("b1rep", [P, HF], F32, isOutput=False)
    b2rep_in = nc.declare_dram_parameter("b2rep", [P, OUTF], F32, isOutput=False)
    idx_in = nc.declare_dram_parameter("idx", [P, s_tiles * 8], I16, isOutput=False)
    out_ext = nc.declare_dram_parameter("out", [NPC, OUTF], F32, isOutput=True)

    with tile.TileContext(nc) as tc:
        with (
            tc.tile_pool(name="const", bufs=1) as cp,
            tc.tile_pool(name="dram", bufs=1, space="DRAM") as dp,
            tc.tile_pool(name="dense_sb", bufs=2) as dsb,
            tc.tile_pool(name="gbuf", bufs=3) as gp,
            tc.tile_pool(name="ehbuf", bufs=2) as ep,
            tc.tile_pool(name="small", bufs=2) as sp,
            tc.tile_pool(name="dense_ps", bufs=2, space="PSUM") as dps,
            tc.tile_pool(name="agg_ps", bufs=2, space="PSUM") as aps,
            tc.tile_pool(name="ad_ps", bufs=1, space="PSUM") as adps,
            tc.tile_pool(name="tr_ps", bufs=1, space="PSUM") as tps,
        ):
            # ---- constants ----
            waug1 = cp.tile([IN_F, D1 + 16], F32)
            nc.sync.dma_start(waug1[:], waug1_in[:])
            w2aug = cp.tile([HF, D2 + 16], F32)
            nc.sync.dma_start(w2aug[:], w2aug_in[:])
            ident = cp.tile([P, P], F32)
            nc.sync.dma_start(ident[:], ident_in[:])
            b1rep = cp.tile([P, HF], F32)
            nc.sync.dma_start(b1rep[:], b1rep_in[:])
            b2rep = cp.tile([P, OUTF], F32)
            nc.sync.dma_start(b2rep[:], b2rep_in[:])
            idx = cp.tile([P, s_tiles * 8], I16)
            nc.sync.dma_start(idx[:], idx_in[:])
            xtown = cp.tile([IN_F, NPC], F32)
            nc.sync.dma_start(xtown[:], xtown_in[:])
            x2town = cp.tile([HF, NPC], F32)   # built in layer-1 gather phase

            table1 = dp.tile([NROWS, T1C], F32)
            table2 = dp.tile([NROWS, T2C], F32)
            x2shard = dp.tile([HF, NPC], F32)
            x2all = dp.tile([NCORE * HF, NPC], F32)

            # dummy rows: h = 0, att_src = -1e9
            zrow = cp.tile([1, T1C], F32)
            nc.vector.memset(zrow[:1, :], 0.0)
            nc.vector.memset(zrow[:1, D1:D1 + 8], BIGNEG)
            nc.sync.dma_start(table1[0:1, :], zrow[:1, :])
            nc.sync.dma_start(table1[NPAD + 1:NPAD + 2, :], zrow[:1, :])
            zrow2 = cp.tile([1, T2C], F32)
            nc.vector.memset(zrow2[:1, :], 0.0)
            nc.vector.memset(zrow2[:1, D2:D2 + 8], BIGNEG)
            nc.sync.dma_start(table2[0:1, :], zrow2[:1, :])
            nc.sync.dma_start(table2[NPAD + 1:NPAD + 2, :], zrow2[:1, :])

            def dense_phase(src_dram, waug_sb, dcols, table, tcols):
                """table rows [h | as] for all NPAD nodes from stacked xT."""
                for r in range(NCORE):
                    xch = dsb.tile([IN_F, NPC], F32, tag="xch")
                    nc.sync.dma_start(xch[:], src_dram[r * IN_F:(r + 1) * IN_F, :])
                    for m in range(BPC):
                        p_ = gpos[r][m]
                        hps = dps.tile([P, dcols + 16], F32, tag="hps")
                        nc.tensor.matmul(
                            hps[:], xch[:, m * P:(m + 1) * P],
                            waug_sb[:], start=True, stop=True,
                        )
                        hsb = dsb.tile([P, dcols + 8], F32, tag="hsb")
                        nc.scalar.copy(hsb[:], hps[:, 0:dcols + 8])
                        nc.sync.dma_start(
                            table[1 + p_ * P: 1 + (p_ + 1) * P, 0:dcols + 8],
                            hsb[:],
                        )

            def gather_phase(table, tcols, dcols, xo_sb, waug_sb, layer):
                out_ch = dcols // HEADS
                for lb in range(BPC):
                    klo, khi = K_lo[lb], K_hi[lb]
                    total = klo + khi
                    # per-dst att_dst for this block via a tiny matmul
                    adp = adps.tile([P, 8], F32, tag="adp")
                    nc.tensor.matmul(
                        adp[:], xo_sb[:, lb * P:(lb + 1) * P],
                        waug_sb[:, dcols + 8:dcols + 16], start=True, stop=True,
                    )
                    ad = sp.tile([P, 1, 8], F32, tag="ad")
                    nc.vector.tensor_copy(ad[:, 0, :], adp[:])

                    dsum = sp.tile([P, 8], F32, tag="dsum")
                    nc.vector.memset(dsum[:], 0.0)
                    acc = aps.tile([P, dcols], F32, tag="acc")
                    if total == 0:
                        nc.vector.memset(acc[:], 0.0)

                    mm = 0
                    for half, khalf in ((0, klo), (1, khi)):
                        src_ap = table[:, :] if half == 0 else table[HIBASE:NROWS, :]
                        t_base = blk_off[lb] + (0 if half == 0 else klo)
                        for c0 in range(0, khalf, CH):
                            k = min(CH, khalf - c0)
                            col0 = t_base + c0
                            g = gp.tile([P, CH, tcols], F32, tag="g")
                            nc.gpsimd.dma_gather(
                                out_ap=g[:, 0:k, :],
                                in_ap=src_ap,
                                idxs_ap=idx[:, col0 * 8:(col0 + k) * 8],
                                num_idxs=k * P,
                                num_idxs_reg=k * P,
                                elem_size=tcols,
                            )
                            lg = sp.tile([P, CH, 8], F32, tag="lg")
                            nc.vector.tensor_tensor(
                                lg[:, 0:k, :],
                                g[:, 0:k, dcols:dcols + 8],
                                ad[:, 0:1, :].broadcast_to((P, k, 8)),
                                mybir.AluOpType.add,
                            )
                            pl = sp.tile([P, CH, 8], F32, tag="pl")
                            nc.scalar.activation(
                                pl[:, 0:k, :], lg[:, 0:k, :],
                                mybir.ActivationFunctionType.Prelu, alpha=NEG,
                            )
                            ex = sp.tile([P, CH, 8], F32, tag="ex")
                            nc.scalar.activation(
                                ex[:, 0:k, :], pl[:, 0:k, :],
                                mybir.ActivationFunctionType.Exp,
                            )
                            red = sp.tile([P, 8], F32, tag="red")
                            nc.vector.tensor_reduce(
                                red[:], ex[:, 0:k, :].rearrange("p t h -> p h t"),
                                axis=mybir.AxisListType.X, op=mybir.AluOpType.add,
                            )
                            nc.vector.tensor_tensor(
                                dsum[:], dsum[:], red[:], mybir.AluOpType.add,
                            )
                            eh = ep.tile([P, CH, dcols], F32, tag="eh")
                            nc.vector.tensor_tensor(
                                eh[:, 0:k, :].rearrange("p t (h w) -> p t h w", w=out_ch),
                                g[:, 0:k, 0:dcols].rearrange("p t (h w) -> p t h w", w=out_ch),
                                ex[:, 0:k, :].to_broadcast([P, k, 8, out_ch]),
                                mybir.AluOpType.mult,
                            )
                            for t in range(k):
                                nc.tensor.matmul(
                                    acc[:], ident[:], eh[:, t, :],
                                    start=(mm == 0), stop=(mm == total - 1),
                                )
                                mm += 1

                    # ---- block epilogue ----
                    tmp8 = sp.tile([P, 8], F32, tag="tmp8")
                    nc.vector.tensor_scalar(
                        tmp8[:], dsum[:], float(HEADS), float(HEADS) * EPS,
                        mybir.AluOpType.mult, mybir.AluOpType.add,
                    )
                    rec = sp.tile([P, 8], F32, tag="rec")
                    nc.vector.reciprocal(rec[:], tmp8[:])
                    onrm = sp.tile([P, dcols], F32, tag="onrm")
                    nc.vector.tensor_tensor(
                        onrm[:].rearrange("p (h w) -> p h w", w=out_ch),
                        acc[:].rearrange("p (h w) -> p h w", w=out_ch),
                        rec[:].to_broadcast([P, 8, out_ch]),
                        mybir.AluOpType.mult,
                    )
                    osum = sp.tile([P, out_ch], F32, tag="osum")
                    nc.vector.tensor_reduce(
                        osum[:], onrm[:].rearrange("p (h w) -> p w h", w=out_ch),
                        axis=mybir.AxisListType.X, op=mybir.AluOpType.add,
                    )
                    if layer == 1:
                        xb = sp.tile([P, HF], F32, tag="xb")
                        nc.vector.tensor_tensor(
                            xb[:], osum[:], b1rep[:], mybir.AluOpType.add,
                        )
                        x2 = sp.tile([P, HF], F32, tag="x2")
                        nc.scalar.activation(
                            x2[:], xb[:], mybir.ActivationFunctionType.Relu,
                        )
                        x2tp = tps.tile([HF, P], F32, tag="x2tp")
                        nc.tensor.transpose(x2tp[:], x2[:], ident[:])
                        nc.scalar.copy(x2town[:, lb * P:(lb + 1) * P], x2tp[:])
                    else:
                        lgt = sp.tile([P, OUTF], F32, tag="lgt")
                        nc.vector.tensor_tensor(
                            lgt[:], osum[:], b2rep[:], mybir.AluOpType.add,
                        )
                        mx = sp.tile([P, 1], F32, tag="mx")
                        nc.vector.tensor_reduce(
                            mx[:], lgt[:], axis=mybir.AxisListType.X,
                            op=mybir.AluOpType.max,
                        )
                        sh = sp.tile([P, OUTF], F32, tag="sh")
                        nc.vector.tensor_scalar(
                            sh[:], lgt[:], mx[:, 0:1], None, mybir.AluOpType.subtract,
                        )
                        ex3 = sp.tile([P, OUTF], F32, tag="ex3")
                        se = sp.tile([P, 1], F32, tag="se")
                        nc.scalar.activation(
                            ex3[:], sh[:], mybir.ActivationFunctionType.Exp,
                            accum_out=se[:],
                        )
                        ln = sp.tile([P, 1], F32, tag="ln")
                        nc.scalar.activation(
                            ln[:], se[:], mybir.ActivationFunctionType.Ln,
                        )
                        res = sp.tile([P, OUTF], F32, tag="res")
                        nc.vector.tensor_scalar(
                            res[:], sh[:], ln[:, 0:1], None, mybir.AluOpType.subtract,
                        )
                        nc.sync.dma_start(out_ext[lb * P:(lb + 1) * P, :], res[:])

            # ===== layer 1 =====
            dense_phase(xt_in, waug1, D1, table1, T1C)
            gather_phase(table1, T1C, D1, xtown, waug1, layer=1)
            nc.sync.dma_start(x2shard[:], x2town[:])
            nc.gpsimd.collective_compute(
                "AllGather",
                mybir.AluOpType.bypass,
                replica_groups=[list(range(NCORE))],
                ins=[x2shard.opt()],
                outs=[x2all.opt()],
            )
            # ===== layer 2 =====
            dense_phase(x2all, w2aug, D2, table2, T2C)
            gather_phase(table2, T2C, D2, x2town, w2aug, layer=2)

    nc.compile()
    return nc


def kernel(x, edge_index, W1, att_src1, att_dst1, b1, W2, att_src2, att_dst2, b2):
    x_pad, xT_stack, idx_maps, new_of_old, sched = _preprocess(x, edge_index)

    waug1 = _augment(W1, att_src1, att_dst1, HEADS, HF)
    w2aug = _augment(W2, att_src2, att_dst2, HEADS, OUTF)
    b1 = np.asarray(b1, dtype=np.float32)
    b2 = np.asarray(b2, dtype=np.float32)

    nc = _build_nc(sched)

    shared = {
        "xt": xT_stack,
        "waug1": waug1,
        "w2aug": w2aug,
        "ident": np.eye(P, dtype=np.float32),
        "b1rep": np.tile(b1[None, :], (P, 1)).astype(np.float32),
        "b2rep": np.tile(b2[None, :], (P, 1)).astype(np.float32),
    }
    in_maps = []
    for c in range(NCORE):
        m = dict(shared)
        m["idx"] = idx_maps[c]
        m["xtown"] = np.ascontiguousarray(
            xT_stack[c * IN_F:(c + 1) * IN_F, :]
        )
        in_maps.append(m)

    res = run_bass_kernel_spmd(nc, in_maps, list(range(NCORE)))
    out_pad = np.concatenate([res.results[c]["out"] for c in range(NCORE)], axis=0)
    return out_pad[new_of_old].astype(np.float32)


# revision 23
# speedup vs baseline: 12.3023x; 12.3023x over previous
"""Two-layer GAT on 8 Trainium2 NeuronCores.

Strategy (graph/data parallel, dst-ownership):
- Host: add self-loops, sort nodes by in-degree (desc), pad to 50176 nodes =
  392 blocks of 128; block b -> core b%8 so per-core degree distribution is
  balanced; new node ids are core-major so each core owns a contiguous range.
- Edges routed to the dst-owner core, stored as a padded ELL structure per
  128-dst block (degree sorting keeps padding ~10%).  Edge slots are split
  into "low"/"high" halves by src table id (dma_gather indices are int16).
- Device: per layer, a dense phase computes table rows [h | att_src] for all
  nodes (redundantly on every core) with one augmented matmul; the gather
  phase dma_gathers the per-edge src rows, computes the segment softmax
  (no max-subtraction needed: logits are O(10)) and aggregates with
  identity-weight matmuls accumulating in PSUM; normalization by the segment
  denominator happens once per dst block after aggregation.
- Between layers the per-core x2 shards (already transposed on device) are
  AllGathered.  Final log_softmax on device; host inverse-permutes rows.

Self-contained: only needs numpy + the concourse (bass) runtime.
"""

import numpy as np

import concourse.bass as bass
import concourse.mybir as mybir
import concourse.tile as tile
from concourse import bacc
from concourse.bass_utils import run_bass_kernel_spmd

# problem constants (hardcoded per spec nn_GAT_19318762897898)
N = 50000
IN_F = 32
HF = 32
OUTF = 16
HEADS = 8
NEG = 0.2
EPS = 1e-16

NCORE = 8
P = 128
BPC = 49                  # blocks per core
NPC = BPC * P             # 6272 nodes per core
NPAD = NCORE * NPC        # 50176
NBLK = NPAD // P          # 392
D1 = HEADS * HF           # 256
D2 = HEADS * OUTF         # 128
T1C = 320                 # table1 row, f32 (1280B: [h 256 | as 8 | pad])
T2C = 192                 # table2 row, f32 (768B:  [h2 128 | as2 8 | pad])
HIBASE = 32768
NROWS = NPAD + 2          # row 0: low dummy; row NPAD+1: high dummy
DUMHI = NPAD + 1 - HIBASE
BIGNEG = -1.0e9
CH = 8                    # gather chunk size in 128-slot tiles

F32 = mybir.dt.float32
I16 = mybir.dt.int16

# ablation toggles (sim experiments only; all False in production)
DBG_DENSE_NO_MM = False
DBG_DENSE_NO_COPY = False
DBG_DENSE_NO_DMA = False
DBG_G_NO_GATHER = False
DBG_G_NO_SOFTMAX = False
DBG_G_NO_EXH = False
DBG_G_NO_MM = False
DBG_G_NO_EPI = False


def _preprocess(x, edge_index):
    """Permute + pack nodes, build per-core ELL gather indices.

    Two independent node labelings:
    - table position `pos` (0..NPAD-1): table row id = 1+pos; the int16
      low/high gather split is pos <= 32766.  Low/high membership is fixed
      first (by degree rank), then nodes are sorted within each half by
      their (low,high) in-edge counts so each 128-dst block has homogeneous
      counts (tight ELL padding).
    - ownership id `new_id` (core-major): blocks of 128 positions are dealt
      to (core, local-block) slots sorted by their (K_lo, K_hi) profile so
      the SPMD-uniform per-block tile counts stay tight across cores.
    """
    src0 = np.asarray(edge_index[0], dtype=np.int64)
    dst0 = np.asarray(edge_index[1], dtype=np.int64)
    loops = np.arange(N, dtype=np.int64)
    src = np.concatenate([src0, loops])
    dst = np.concatenate([dst0, loops])

    # stage 1: low/high membership by degree rank
    deg = np.bincount(dst, minlength=N)
    rank_of_old = np.empty(N, dtype=np.int64)
    rank_of_old[np.argsort(-deg, kind="stable")] = np.arange(N)
    is_lo = rank_of_old <= (HIBASE - 2)               # table id 1+pos <= 32767

    cl = np.bincount(dst[is_lo[src]], minlength=N)    # per-dst low in-edges
    chh = np.bincount(dst[~is_lo[src]], minlength=N)

    # stage 2: sort within each half by (cl, ch) desc -> table positions
    lo_nodes = np.flatnonzero(is_lo)
    hi_nodes = np.flatnonzero(~is_lo)
    lo_sorted = lo_nodes[np.lexsort((-chh[lo_nodes], -cl[lo_nodes]))]
    hi_sorted = hi_nodes[np.lexsort((-chh[hi_nodes], -cl[hi_nodes]))]
    pos_of_old = np.empty(N, dtype=np.int64)
    pos_of_old[lo_sorted] = np.arange(len(lo_sorted))
    pos_of_old[hi_sorted] = (HIBASE - 1) + np.arange(len(hi_sorted))

    # per-position counts; per-block maxima
    clp = np.zeros(NPAD, np.int64)
    chp = np.zeros(NPAD, np.int64)
    clp[pos_of_old] = cl
    chp[pos_of_old] = chh
    maxlo_g = clp.reshape(NBLK, P).max(1)             # [392]
    maxhi_g = chp.reshape(NBLK, P).max(1)

    # deal: block g -> core g%8, local block g//8.  Consecutive blocks have
    # similar count profiles (positions are count-sorted), so the per-lb max
    # over cores stays tight, and gpos[r][m] = m*8+r is a regular stride the
    # dense phase exploits for batched contiguous table writes.
    core_of_g = np.arange(NBLK) % NCORE
    lb_of_g = np.arange(NBLK) // NCORE
    gpos = np.empty((NCORE, BPC), np.int64)
    gpos[core_of_g, lb_of_g] = np.arange(NBLK)

    # ownership ids
    g_of_old = pos_of_old // P
    j_of_old = pos_of_old % P
    new_of_old = core_of_g[g_of_old] * NPC + lb_of_g[g_of_old] * P + j_of_old

    x = np.asarray(x, dtype=np.float32)
    x_pad = np.zeros((NPAD, IN_F), np.float32)
    x_pad[new_of_old] = x

    tid = 1 + pos_of_old[src]                         # table row ids, 1..NPAD
    hi = tid > (HIBASE - 1)
    dst_n = new_of_old[dst]                           # ownership id of dst

    # rank of each edge within its (dst, half) group
    key = dst_n * 2 + hi
    o = np.argsort(key, kind="stable")
    ks = key[o]
    newrun = np.r_[True, ks[1:] != ks[:-1]]
    run_start = np.flatnonzero(newrun)
    run_len = np.diff(np.r_[run_start, len(ks)])
    rank_sorted = np.arange(len(ks), dtype=np.int64) - np.repeat(run_start, run_len)
    rank = np.empty(len(ks), np.int64)
    rank[o] = rank_sorted

    # uniform per-local-block tile counts (max over the 8 dealt blocks)
    K_lo = maxlo_g.reshape(BPC, NCORE).max(1)
    K_hi = maxhi_g.reshape(BPC, NCORE).max(1)

    blk_tiles = K_lo + K_hi
    blk_off = np.r_[0, np.cumsum(blk_tiles)[:-1]]
    s_tiles = int(blk_tiles.sum())
    slots = s_tiles * P

    # defaults: low tiles gather row 0, high tiles gather row NPAD+1
    tile_is_hi = np.zeros(s_tiles, dtype=bool)
    for lb in range(BPC):
        t0 = blk_off[lb]
        tile_is_hi[t0 + K_lo[lb]: t0 + K_lo[lb] + K_hi[lb]] = True
    default = np.where(tile_is_hi, DUMHI, 0).astype(np.int16)
    base = np.repeat(default, P)                      # [slots]

    core_e = dst_n // NPC
    lb_e = (dst_n % NPC) // P
    j_e = dst_n % P
    t_in_blk = np.where(hi, K_lo[lb_e] + rank, rank)
    slot_e = (blk_off[lb_e] + t_in_blk) * P + j_e
    val_e = np.where(hi, tid - HIBASE, tid).astype(np.int16)

    idx_maps = []
    for c in range(NCORE):
        arr = base.copy()
        m = core_e == c
        arr[slot_e[m]] = val_e[m]
        wrapped = arr.reshape(-1, 16).T.copy()        # [16, slots/16]
        idx_maps.append(np.tile(wrapped, (NCORE, 1)))  # [128, slots/16]

    # xT stacked rank-major: [NCORE*IN_F, NPC]
    xT = np.ascontiguousarray(x_pad.T)                # [32, NPAD]
    xT_stack = np.concatenate(
        [xT[:, c * NPC:(c + 1) * NPC] for c in range(NCORE)], axis=0
    )                                                 # [256, NPC]

    sched = dict(K_lo=[int(v) for v in K_lo], K_hi=[int(v) for v in K_hi],
                 blk_off=[int(v) for v in blk_off], s_tiles=s_tiles, slots=slots,
                 gpos=[[int(v) for v in row] for row in gpos])
    return x_pad, xT_stack, idx_maps, new_of_old, sched


def _augment(W, att_src, att_dst, heads, out_ch, tcols):
    W = np.asarray(W, dtype=np.float32)
    a_s = np.asarray(att_src, dtype=np.float32)
    a_d = np.asarray(att_dst, dtype=np.float32)
    Wr = W.reshape(IN_F if W.shape[0] == IN_F else HF, heads, out_ch)
    was = np.einsum("khc,hc->kh", Wr, a_s)
    wad = np.einsum("khc,hc->kh", Wr, a_d)
    out = np.zeros((W.shape[0], tcols), np.float32)
    out[:, :W.shape[1]] = W
    out[:, W.shape[1]:W.shape[1] + heads] = was
    out[:, W.shape[1] + heads:W.shape[1] + 2 * heads] = wad
    return out


def _build_nc(sched, phases=("d1", "g1", "ag", "d2", "g2")):
    K_lo = sched["K_lo"]
    K_hi = sched["K_hi"]
    blk_off = sched["blk_off"]
    s_tiles = sched["s_tiles"]
    gpos = sched["gpos"]

    nc = bacc.Bacc(None, target_bir_lowering=False, debug=False)

    xt_in = nc.declare_dram_parameter("xt", [NCORE * IN_F, NPC], F32, isOutput=False)
    xtown_in = nc.declare_dram_parameter("xtown", [IN_F, NPC], F32, isOutput=False)
    waug1_in = nc.declare_dram_parameter("waug1", [IN_F, T1C], F32, isOutput=False)
    w2aug_in = nc.declare_dram_parameter("w2aug", [HF, T2C], F32, isOutput=False)
    ident_in = nc.declare_dram_parameter("ident", [P, P], F32, isOutput=False)
    b1rep_in = nc.declare_dram_parameter("b1rep", [P, HF], F32, isOutput=False)
    b2rep_in = nc.declare_dram_parameter("b2rep", [P, OUTF], F32, isOutput=False)
    idx_in = nc.declare_dram_parameter("idx", [P, s_tiles * 8], I16, isOutput=False)
    out_ext = nc.declare_dram_parameter("out", [NPC, OUTF], F32, isOutput=True)

    with tile.TileContext(nc) as tc:
        with (
            tc.tile_pool(name="const", bufs=1) as cp,
            tc.tile_pool(name="dram", bufs=1, space="DRAM") as dp,
            tc.tile_pool(name="dense_sb", bufs=2) as dsb,
            tc.tile_pool(name="gbuf", bufs=3) as gp,
            tc.tile_pool(name="ehbuf", bufs=2) as ep,
            tc.tile_pool(name="small", bufs=2) as sp,
            tc.tile_pool(name="dense_ps", bufs=2, space="PSUM") as dps,
            tc.tile_pool(name="agg_ps", bufs=2, space="PSUM") as aps,
            tc.tile_pool(name="ad_ps", bufs=1, space="PSUM") as adps,
            tc.tile_pool(name="tr_ps", bufs=1, space="PSUM") as tps,
        ):
            # ---- constants ----
            waug1 = cp.tile([IN_F, T1C], F32)
            nc.sync.dma_start(waug1[:], waug1_in[:])
            w2aug = cp.tile([HF, T2C], F32)
            nc.sync.dma_start(w2aug[:], w2aug_in[:])
            ident = cp.tile([P, P], F32)
            nc.sync.dma_start(ident[:], ident_in[:])
            b1rep = cp.tile([P, HF], F32)
            nc.sync.dma_start(b1rep[:], b1rep_in[:])
            b2rep = cp.tile([P, OUTF], F32)
            nc.sync.dma_start(b2rep[:], b2rep_in[:])
            idx = cp.tile([P, s_tiles * 8], I16)
            nc.sync.dma_start(idx[:], idx_in[:])
            xtown = cp.tile([IN_F, NPC], F32)
            nc.sync.dma_start(xtown[:], xtown_in[:])
            x2town = cp.tile([HF, NPC], F32)   # built in layer-1 gather phase

            table1 = dp.tile([NROWS, T1C], F32)
            table2 = dp.tile([NROWS, T2C], F32)
            x2shard = dp.tile([HF, NPC], F32)
            x2all = dp.tile([NCORE * HF, NPC], F32)

            # dummy rows: h = 0, att_src = -1e9
            zrow = cp.tile([1, T1C], F32)
            nc.vector.memset(zrow[:1, :], 0.0)
            nc.vector.memset(zrow[:1, D1:D1 + 8], BIGNEG)
            nc.sync.dma_start(table1[0:1, :], zrow[:1, :])
            nc.sync.dma_start(table1[NPAD + 1:NPAD + 2, :], zrow[:1, :])
            zrow2 = cp.tile([1, T2C], F32)
            nc.vector.memset(zrow2[:1, :], 0.0)
            nc.vector.memset(zrow2[:1, D2:D2 + 8], BIGNEG)
            nc.sync.dma_start(table2[0:1, :], zrow2[:1, :])
            nc.sync.dma_start(table2[NPAD + 1:NPAD + 2, :], zrow2[:1, :])

            def dense_phase(src_dram, waug_sb, dcols, table, tcols):
                """table rows [h | as | 0...] for all NPAD nodes from stacked xT.

                Walks core-major (r, m); with gpos[r][m] = m*8+r, batches of DB
                local blocks write one strided full-row DMA (table blocks
                m*8+r, (m+1)*8+r, ... are 1024 rows apart — a regular AP).
                """
                DB = 7
                # strided view: [q=core, p, m=local block, f]
                tview = table[1:1 + NBLK * P, :].rearrange(
                    "(m q p) f -> q p m f", q=NCORE, p=P)
                for r in range(NCORE):
                    xch = dsb.tile([IN_F, NPC], F32, tag="xch")
                    nc.sync.dma_start(xch[:], src_dram[r * IN_F:(r + 1) * IN_F, :])
                    for m0 in range(0, BPC, DB):
                        db = min(DB, BPC - m0)
                        hsb = dsb.tile([P, DB, tcols], F32, tag="hsb")
                        for t in range(db):
                            m_ = m0 + t
                            hps = dps.tile([P, tcols], F32, tag="hps")
                            if not DBG_DENSE_NO_MM:
                                nc.tensor.matmul(
                                    hps[:], xch[:, m_ * P:(m_ + 1) * P],
                                    waug_sb[:], start=True, stop=True,
                                )
                            if not DBG_DENSE_NO_COPY:
                                nc.scalar.copy(hsb[:, t, :], hps[:])
                        if not DBG_DENSE_NO_DMA:
                            nc.sync.dma_start(
                                tview[r, :, m0:m0 + db, :],
                                hsb[:, 0:db, :].rearrange("p t f -> p t f"),
                            )

            def gather_phase(table, tcols, dcols, xo_sb, waug_sb, layer):
                out_ch = dcols // HEADS
                for lb in range(BPC):
                    klo, khi = K_lo[lb], K_hi[lb]
                    total = klo + khi
                    # per-dst att_dst for this block via a tiny matmul
                    adp = adps.tile([P, 8], F32, tag="adp")
                    nc.tensor.matmul(
                        adp[:], xo_sb[:, lb * P:(lb + 1) * P],
                        waug_sb[:, dcols + 8:dcols + 16], start=True, stop=True,
                    )
                    ad = sp.tile([P, 1, 8], F32, tag="ad")
                    nc.vector.tensor_copy(ad[:, 0, :], adp[:])

                    dsum = sp.tile([P, 8], F32, tag="dsum")
                    nc.vector.memset(dsum[:], 0.0)
                    acc = aps.tile([P, dcols], F32, tag="acc")
                    if total == 0:
                        nc.vector.memset(acc[:], 0.0)

                    no_mm = DBG_G_NO_MM or DBG_G_NO_EXH or DBG_G_NO_SOFTMAX or DBG_G_NO_GATHER
                    no_exh = DBG_G_NO_EXH or DBG_G_NO_SOFTMAX or DBG_G_NO_GATHER
                    no_sm = DBG_G_NO_SOFTMAX or DBG_G_NO_GATHER
                    if no_mm and total > 0:
                        nc.vector.memset(acc[:], 0.0)
                    mm = 0
                    for half, khalf in ((0, klo), (1, khi)):
                        src_ap = table[:, :] if half == 0 else table[HIBASE:NROWS, :]
                        t_base = blk_off[lb] + (0 if half == 0 else klo)
                        for c0 in range(0, khalf, CH):
                            k = min(CH, khalf - c0)
                            col0 = t_base + c0
                            g = gp.tile([P, CH, tcols], F32, tag="g")
                            if not DBG_G_NO_GATHER:
                                nc.gpsimd.dma_gather(
                                    out_ap=g[:, 0:k, :],
                                    in_ap=src_ap,
                                    idxs_ap=idx[:, col0 * 8:(col0 + k) * 8],
                                    num_idxs=k * P,
                                    num_idxs_reg=k * P,
                                    elem_size=tcols,
                                )
                            if not no_sm:
                                lg = sp.tile([P, CH, 8], F32, tag="lg")
                                nc.vector.tensor_tensor(
                                    lg[:, 0:k, :],
                                    g[:, 0:k, dcols:dcols + 8],
                                    ad[:, 0:1, :].broadcast_to((P, k, 8)),
                                    mybir.AluOpType.add,
                                )
                                pl = sp.tile([P, CH, 8], F32, tag="pl")
                                nc.scalar.activation(
                                    pl[:, 0:k, :], lg[:, 0:k, :],
                                    mybir.ActivationFunctionType.Prelu, alpha=NEG,
                                )
                                ex = sp.tile([P, CH, 8], F32, tag="ex")
                                nc.scalar.activation(
                                    ex[:, 0:k, :], pl[:, 0:k, :],
                                    mybir.ActivationFunctionType.Exp,
                                )
                                red = sp.tile([P, 8], F32, tag="red")
                                nc.vector.tensor_reduce(
                                    red[:], ex[:, 0:k, :].rearrange("p t h -> p h t"),
                                    axis=mybir.AxisListType.X, op=mybir.AluOpType.add,
                                )
                                nc.vector.tensor_tensor(
                                    dsum[:], dsum[:], red[:], mybir.AluOpType.add,
                                )
                            if not no_exh:
                                eh = ep.tile([P, CH, dcols], F32, tag="eh")
                                nc.vector.tensor_tensor(
                                    eh[:, 0:k, :].rearrange("p t (h w) -> p t h w", w=out_ch),
                                    g[:, 0:k, 0:dcols].rearrange("p t (h w) -> p t h w", w=out_ch),
                                    ex[:, 0:k, :].to_broadcast([P, k, 8, out_ch]),
                                    mybir.AluOpType.mult,
                                )
                            if not no_mm:
                                for t in range(k):
                                    nc.tensor.matmul(
                                        acc[:], ident[:], eh[:, t, :],
                                        start=(mm == 0), stop=(mm == total - 1),
                                    )
                                    mm += 1

                    # ---- block epilogue ----
                    tmp8 = sp.tile([P, 8], F32, tag="tmp8")
                    nc.vector.tensor_scalar(
                        tmp8[:], dsum[:], float(HEADS), float(HEADS) * EPS,
                        mybir.AluOpType.mult, mybir.AluOpType.add,
                    )
                    rec = sp.tile([P, 8], F32, tag="rec")
                    nc.vector.reciprocal(rec[:], tmp8[:])
                    onrm = sp.tile([P, dcols], F32, tag="onrm")
                    nc.vector.tensor_tensor(
                        onrm[:].rearrange("p (h w) -> p h w", w=out_ch),
                        acc[:].rearrange("p (h w) -> p h w", w=out_ch),
                        rec[:].to_broadcast([P, 8, out_ch]),
                        mybir.AluOpType.mult,
                    )
                    osum = sp.tile([P, out_ch], F32, tag="osum")
                    nc.vector.tensor_reduce(
                        osum[:], onrm[:].rearrange("p (h w) -> p w h", w=out_ch),
                        axis=mybir.AxisListType.X, op=mybir.AluOpType.add,
                    )
                    if layer == 1:
                        xb = sp.tile([P, HF], F32, tag="xb")
                        nc.vector.tensor_tensor(
                            xb[:], osum[:], b1rep[:], mybir.AluOpType.add,
                        )
                        x2 = sp.tile([P, HF], F32, tag="x2")
                        nc.scalar.activation(
                            x2[:], xb[:], mybir.ActivationFunctionType.Relu,
                        )
                        x2tp = tps.tile([HF, P], F32, tag="x2tp")
                        nc.tensor.transpose(x2tp[:], x2[:], ident[:])
                        nc.scalar.copy(x2town[:, lb * P:(lb + 1) * P], x2tp[:])
                    else:
                        lgt = sp.tile([P, OUTF], F32, tag="lgt")
                        nc.vector.tensor_tensor(
                            lgt[:], osum[:], b2rep[:], mybir.AluOpType.add,
                        )
                        mx = sp.tile([P, 1], F32, tag="mx")
                        nc.vector.tensor_reduce(
                            mx[:], lgt[:], axis=mybir.AxisListType.X,
                            op=mybir.AluOpType.max,
                        )
                        sh = sp.tile([P, OUTF], F32, tag="sh")
                        nc.vector.tensor_scalar(
                            sh[:], lgt[:], mx[:, 0:1], None, mybir.AluOpType.subtract,
                        )
                        ex3 = sp.tile([P, OUTF], F32, tag="ex3")
                        se = sp.tile([P, 1], F32, tag="se")
                        nc.scalar.activation(
                            ex3[:], sh[:], mybir.ActivationFunctionType.Exp,
                            accum_out=se[:],
                        )
                        ln = sp.tile([P, 1], F32, tag="ln")
                        nc.scalar.activation(
                            ln[:], se[:], mybir.ActivationFunctionType.Ln,
                        )
                        res = sp.tile([P, OUTF], F32, tag="res")
                        nc.vector.tensor_scalar(
                            res[:], sh[:], ln[:, 0:1], None, mybir.AluOpType.subtract,
                        )
                        nc.sync.dma_start(out_ext[lb * P:(lb + 1) * P, :], res[:])

            # ===== layer 1 =====
            if "d1" in phases:
                dense_phase(xt_in, waug1, D1, table1, T1C)
            if "g1" in phases:
                gather_phase(table1, T1C, D1, xtown, waug1, layer=1)
            if "ag" in phases:
                nc.sync.dma_start(x2shard[:], x2town[:])
                nc.gpsimd.collective_compute(
                    "AllGather",
                    mybir.AluOpType.bypass,
                    replica_groups=[list(range(NCORE))],
                    ins=[x2shard.opt()],
                    outs=[x2all.opt()],
                )
            # ===== layer 2 =====
            if "d2" in phases:
                dense_phase(x2all, w2aug, D2, table2, T2C)
            if "g2" in phases:
                gather_phase(table2, T2C, D2, x2town, w2aug, layer=2)

    nc.compile()
    return nc


def kernel(x, edge_index, W1, att_src1, att_dst1, b1, W2, att_src2, att_dst2, b2):
    x_pad, xT_stack, idx_maps, new_of_old, sched = _preprocess(x, edge_index)

    waug1 = _augment(W1, att_src1, att_dst1, HEADS, HF, T1C)
    w2aug = _augment(W2, att_src2, att_dst2, HEADS, OUTF, T2C)
    b1 = np.asarray(b1, dtype=np.float32)
    b2 = np.asarray(b2, dtype=np.float32)

    nc = _build_nc(sched)

    shared = {
        "xt": xT_stack,
        "waug1": waug1,
        "w2aug": w2aug,
        "ident": np.eye(P, dtype=np.float32),
        "b1rep": np.tile(b1[None, :], (P, 1)).astype(np.float32),
        "b2rep": np.tile(b2[None, :], (P, 1)).astype(np.float32),
    }
    in_maps = []
    for c in range(NCORE):
        m = dict(shared)
        m["idx"] = idx_maps[c]
        m["xtown"] = np.ascontiguousarray(
            xT_stack[c * IN_F:(c + 1) * IN_F, :]
        )
        in_maps.append(m)

    res = run_bass_kernel_spmd(nc, in_maps, list(range(NCORE)))
    out_pad = np.concatenate([res.results[c]["out"] for c in range(NCORE)], axis=0)
    return out_pad[new_of_old].astype(np.float32)
